# revision 19
# baseline (speedup 1.0000x reference)
"""MoE gating kernel for Trainium2 (Bass/Tile), data-parallel over 8 NeuronCores.

Computes: logits = x @ W_g.T ; top-2 values; softmax over the 2 values.
  p1 = sigmoid(v1 - v2), p2 = sigmoid(v2 - v1)  (v1 >= v2 the top-2 logits)

Sharding: tokens split 8 ways (2048 tokens/core), W_g replicated.

The kernel is roofline-bound on the 16.8MB/core HBM read of x (~44us with all
8 cores sharing the chip's HBM). Everything else hides inside that window:
  - x is cast f32->fp16 *during* the SWDGE DMA (read side runs at full HBM
    rate; SBUF write bytes halve); W_g loads f32 on the concurrent HWDGE
    queue and is cast on the idle DVE
  - PE transposes x fp16 -> PSUM stays fp16 (is_transpose keeps dtype), so
    DVE drains run in 2x_1P packed mode; fp16 also enables FWL so the
    transposes stream at ~56ns/tile
  - logits are computed TOKEN-MAJOR: xT_k is the (FWL) stationary, wgT2_k
    the 64-wide moving operand, accumulating [128 tok, 64 expert] in PSUM.
    Top-2 (DVE MAX8) then reads straight from PSUM: no logits-transpose,
    no staging copy
  - one-group pipeline skew, 1-tile first/last chunks, and outputs staged
    in SBUF with the bulk output DMA deferred past the end of the x stream
    (tiny-descriptor DMAs concurrent with the stream cost it ~25%)
fp16 adds ~1e-3 worst-case abs error on the probabilities; gate is 2e-2.
"""

import sys

sys.path.insert(0, "/opt/trn_rl_repo")

from contextlib import ExitStack

import numpy as np

import concourse.bass as bass
import concourse.bacc as bacc
import concourse.mybir as mybir
from concourse import masks
from concourse.tile import TileContext
from concourse.bass_utils import run_bass_kernel_spmd

TOKENS = 16384
DIM = 2048
E = 64  # num experts
NCORES = 8
TPC = TOKENS // NCORES  # tokens per core
P = 128
KT = DIM // P  # 16 contraction tiles
G = 256  # max token group width

F32 = mybir.dt.float32
F16 = mybir.dt.float16


def _emit(tc: TileContext, ctx: ExitStack, x_ap, wg_ap, out_ap):
    nc = tc.nc

    singles = ctx.enter_context(tc.tile_pool(name="singles", bufs=1))
    xpool = ctx.enter_context(tc.tile_pool(name="xpool", bufs=1))
    xtpool = ctx.enter_context(tc.tile_pool(name="xtpool", bufs=3))
    spool = ctx.enter_context(tc.tile_pool(name="spool", bufs=4))
    psum_t = ctx.enter_context(tc.tile_pool(name="psum_t", bufs=3, space="PSUM"))
    psum_l = ctx.enter_context(tc.tile_pool(name="psum_l", bufs=2, space="PSUM"))

    # --- x streamed in as fp16, one SWDGE cast-DMA per group. 1-tile chunks
    # at both ends shorten the PE ramp-up and the post-stream serial tail.
    TBS = [2, 2, 2, 2, 2, 2, 2, 1, 1]  # tiles per group (sums to 16)
    offs = [sum(TBS[:g]) * P for g in range(len(TBS))]
    xs = []
    for g, tb in enumerate(TBS):
        xg = xpool.tile([P, 2, DIM], F16, tag="x", name="x", bufs=len(TBS))
        xs.append(xg)

    def load_chunk(g):
        r0 = offs[g]
        nc.gpsimd.dma_start(
            out=xs[g][:, 0 : TBS[g], :],
            in_=x_ap[r0 : r0 + TBS[g] * P, :].rearrange("(t p) d -> p t d", p=P),
        )

    # chunk 0 rides the HWDGE queue as f32 (its first byte lands ~1.8us before
    # the SWDGE queue wakes up) and is cast to fp16 on the then-idle DVE; the
    # SWDGE stream starts at chunk 1 concurrently, packing the HBM read
    # schedule ~2us tighter overall.
    xg0_raw = singles.tile([P, 2, DIM], F32)
    nc.sync.dma_start(
        out=xg0_raw[:],
        in_=x_ap[0 : 2 * P, :].rearrange("(t p) d -> p t d", p=P),
    )
    nc.vector.tensor_copy(xs[0][:], xg0_raw[:])
    load_chunk(1)
    # identity early on the Q7 queue: ready before chunk-0's transposes
    identh = singles.tile([P, P], F16)
    masks.make_identity(nc, identh[:])
    load_chunk(2)
    # W_g f32 flat [128, 1024]: row (2e+h) = W_g[e, 1024h:+1024]; HWDGE queue
    # behind chunk 0, cast on the DVE
    wg_raw = singles.tile([P, DIM // 2], F32)
    nc.sync.dma_start(out=wg_raw[:], in_=wg_ap.rearrange("e (h c) -> (e h) c", h=2))
    wg_sb = singles.tile([P, DIM // 2], F16)
    nc.vector.tensor_copy(wg_sb[:], wg_raw[:])
    for g in range(3, len(TBS)):
        load_chunk(g)

    # PE warm-up (HAM clock gate) + ACT sigmoid table preload during DMA wait
    warm = singles.tile([P, P], F16)
    nc.vector.memset(warm[:], 0.0)
    for _ in range(8):
        pw = psum_l.tile([P, E], F32, name="lp", tag="lp", bufs=3)
        nc.tensor.matmul(pw[:], warm[:], warm[:, 0:E])
    sig_warm = spool.tile([P, 2], F32)
    nc.scalar.activation(sig_warm[:], warm[:, 0:2], mybir.ActivationFunctionType.Sigmoid)

    # wgT2[c, k, e] = W_g[e, 128k + c]: contiguous [128 d, 64 e] moving
    # operand per k-tile. PE-transpose wg_sb (pt[c, 2e+h] = W_g[e, 1024h+128j+c])
    # then de-interleave the h-parity during the PSUM drain.
    wgT2 = singles.tile([P, KT, E], F16)

    def build_wgT():
        for j in range(KT // 2):
            pt = psum_l.tile([P, P], F16, name="wg_ps", tag="wg_ps", bufs=2)
            nc.tensor.matmul(
                pt[:], wg_sb[:, j * P : (j + 1) * P], identh[:], is_transpose=True
            )
            for h in range(2):
                src = pt[:, h : h + 1]
                nc.vector.tensor_copy(
                    wgT2[:, 8 * h + j, :],
                    bass.AP(tensor=src.tensor, offset=src.offset, ap=[src.ap[0], [2, E]]),
                )

    def transposes(g, xt, half):
        # x [t,d] -> xT [128 d, k*gw t]; 8 fp16 [128,128] transposes fill one
        # PSUM bank ([128,1024] fp16), drained by one 2x-packed DVE copy.
        xg = xs[g]
        tbs = TBS[g]
        gw = tbs * P
        kq = 8 // tbs  # k-tiles per PSUM bank
        nq = KT // kq  # banks for this group
        for q in range(nq // 2 * half, nq // 2 * (half + 1)):
            pt = psum_t.tile([P, 8 * P], F16)
            for dk in range(kq):
                k = q * kq + dk
                for tb in range(tbs):
                    nc.tensor.matmul(
                        pt[:, dk * gw + tb * P : dk * gw + (tb + 1) * P],
                        xg[:, tb, k * P : (k + 1) * P],
                        identh[:],
                        is_transpose=True,
                    )
            nc.vector.tensor_copy(xt[:, q * kq * gw : (q + 1) * kq * gw], pt[:])

    def mm_blocks(g, xt):
        # logits [128 tok, 64 e] per token block, accumulated over k with
        # xT_k as the FWL stationary and wgT2_k as the 64-wide moving operand
        gw = TBS[g] * P
        lps = []
        for tb in range(TBS[g]):
            lp = psum_l.tile([P, E], F32, name="lp", tag="lp", bufs=3)
            for k in range(KT):
                nc.tensor.matmul(
                    lp[:],
                    xt[:, k * gw + tb * P : k * gw + (tb + 1) * P],
                    wgT2[:, k, :],
                    start=(k == 0),
                    stop=(k == KT - 1),
                )
            lps.append(lp)
        return lps

    # probs staged in SBUF; 8-byte-run output DMAs deferred past the x stream
    obuf = singles.tile([P, (TPC // P) * 2], F32)

    def epilogue(g, lps):
        # top-2 + softmax straight from PSUM logits
        for tb in range(TBS[g]):
            blk = offs[g] // P + tb
            eps = spool.tile([P, 10], F32, name="eps", tag="eps")
            nc.vector.max(out=eps[:, 0:8], in_=lps[tb][:])
            m2 = eps[:, 0:2]
            rev = bass.AP(tensor=m2.tensor, offset=m2.offset + 1, ap=[m2.ap[0], [-1, 2]])
            nc.vector.tensor_sub(eps[:, 8:10], m2, rev)  # [v1-v2, v2-v1]
            nc.scalar.activation(
                obuf[:, blk * 2 : (blk + 1) * 2],
                eps[:, 8:10],
                mybir.ActivationFunctionType.Sigmoid,
            )

    NGV = len(TBS)
    NBLK = TPC // P
    L = NGV - 1
    xts, lpd = {}, {}
    for g in range(NGV - 1):
        xts[g] = xtpool.tile([P, KT * G], F16, name="xt", tag="xt")
        transposes(g, xts[g], 0)
        if g == 0:
            build_wgT()
        if g >= 1:
            lpd[g - 1] = mm_blocks(g - 1, xts[g - 1])
            epilogue(g - 1, lpd.pop(g - 1))
        transposes(g, xts[g], 1)
    # final group: pending mm/epilogue first (they don't need the last chunk),
    # bulk output DMA after the last transposes (past the end of the stream)
    lpd[L - 1] = mm_blocks(L - 1, xts[L - 1])
    epilogue(L - 1, lpd.pop(L - 1))
    xts[L] = xtpool.tile([P, KT * G], F16, name="xt", tag="xt")
    transposes(L, xts[L], 0)
    transposes(L, xts[L], 1)
    cut = offs[L] // P
    nc.sync.dma_start(
        out=out_ap[0 : cut * P, :].rearrange("(b p) c -> p b c", p=P),
        in_=obuf[:, 0 : cut * 2],
    )
    lpd[L] = mm_blocks(L, xts[L])
    epilogue(L, lpd.pop(L))
    nc.scalar.dma_start(
        out=out_ap[cut * P : NBLK * P, :].rearrange("(b p) c -> p b c", p=P),
        in_=obuf[:, cut * 2 : NBLK * 2],
    )


_NC_CACHE = {}


def _build():
    key = "nc"
    if key in _NC_CACHE:
        return _NC_CACHE[key]
    nc = bacc.Bacc(trn_type="TRN2")
    x = nc.dram_tensor("x", [TPC, DIM], F32, kind="ExternalInput")
    wg = nc.dram_tensor("w_g", [E, DIM], F32, kind="ExternalInput")
    out = nc.dram_tensor("out", [TPC, 2], F32, kind="ExternalOutput")
    with TileContext(nc) as tc, ExitStack() as ctx:
        _emit(tc, ctx, x.ap(), wg.ap(), out.ap())
    if not nc.is_finalized():
        nc.finalize()
    _NC_CACHE[key] = nc
    return nc


def _run(x, W_g, trace=False):
    nc = _build()
    x = np.ascontiguousarray(np.asarray(x, dtype=np.float32))
    W_g = np.ascontiguousarray(np.asarray(W_g, dtype=np.float32))
    in_maps = [
        {"x": np.ascontiguousarray(x[c * TPC : (c + 1) * TPC]), "w_g": W_g}
        for c in range(NCORES)
    ]
    res = run_bass_kernel_spmd(nc, in_maps, core_ids=list(range(NCORES)), trace=trace)
    out = np.concatenate([r["out"] for r in res.results], axis=0)
    return out, res


def kernel(x, W_g):
    out, _ = _run(x, W_g, trace=False)
    return out


def kernel_profiled(x, W_g, mm_f32r=True):
    # mm_f32r kept for test.py compatibility; the kernel is fp16-only
    out, res = _run(x, W_g, trace=True)
    return out, res
